# revision 6
# baseline (speedup 1.0000x reference)
"""CRF decode (conv features -> emission scores -> Viterbi) on 8 TRN2 cores.

Data-parallel over the batch: each core gets B/8 = 4096 words (32 tiles of
128 words on partitions). Per core:
  - conv+emission collapse to one (128 -> 26) linear map A = W @ C applied on
    the PE per 128-word tile (one PE transpose + one matmul per letter;
    PSUM->SBUF copies batched 4 letters per Activation instruction, scores
    accumulated 14 letters per PSUM tile -> one copy per tile),
  - Viterbi forward DP on the DVE with words on partitions and a single fused
    custom-DVE pass per (step, tile): pages of 27 (26 prev-labels in reversed
    "primed" order + one -inf pad). An 8-stage hand-assembled uop program
    computes a = T''+v, running page max r (reset at page boundaries via a
    SUB_DIM_DONE step state), achiever flag f = (a >= r), and a select-scan
    k = stream index of the last achiever. Output stream = k, except the pad
    slot which emits r. Page slot 25 is the first-argmax backpointer (exact
    jax tie-breaking via the label reversal), slot 26 the max. This halves
    DVE work vs separate max + argmax passes.
  - the v updates (r + score) and backpointer extraction (k - offset) run on
    the Pool engine (gpsimd) in the DVE's shadow; tiles are processed in
    staggered waves so the DVE never waits on emissions,
  - batched backtrack over all tiles (fused one-hot select custom op EQSEL +
    grouped max per step), final primed->true label flip, int32 convert,
    strided DMA out.
"""

import sys

if "/opt/trn_rl_repo" not in sys.path:
    sys.path.insert(0, "/opt/trn_rl_repo")

import numpy as np

from dataclasses import dataclass
from typing import Any

import concourse.bacc as bacc
import concourse.mybir as mybir
import concourse.tile as tile
from concourse import bass_utils
from concourse import dve_ops
from concourse.dve_ops import DveOp
from concourse.dve_spec import (
    Spec, Src0, Src1, C0, C1, Zero, One, lower, scan, eq, Idx, SubIdx,
    Bin, Tri, Scan, _Stage, _State, _Placement, _assemble, PREV, COUNT_ONCE,
)
from concourse.dve_uop import (
    AluInp, AluOp as UAluOp, DveOpSpec, OutSel, Trigger,
)

F32 = mybir.dt.float32
AX = mybir.AxisListType
OP = mybir.AluOpType
ACT_COPY = mybir.ActivationFunctionType.Copy

B = 32768
M = 14
H, WD = 16, 8
F = 128
L = 26
KS = 5
NCORES = 8
BC = B // NCORES          # words per core
NT = BC // 128            # 128-word tiles per core (32)
NPAD = L + 1              # 27: page width (26 prev-labels + -inf pad)
PAD_VAL = -3e38
RSEED = -1e37             # r scan seed / pad detection threshold

_HAND_CACHE: dict = {}


@dataclass(frozen=True)
class _HandDveOp(DveOp):
    """DveOp whose uops come from `build` (hand-assembled program that the
    Spec language cannot express: two interacting scans + select-routed
    output)."""

    build: Any = None

    def compile(self, ver):
        key = (self.name, ver)
        if (r := _HAND_CACHE.get(key)) is not None:
            return r
        uops = self.build(ver)
        for u in uops:
            u.validate(ver)
        res = DveOpSpec(
            name=self.name, opcode=dve_ops.get_dve_sub_opcode(self.name),
            uops=uops, rd1_en=True)
        _HAND_CACHE[key] = res
        return res


def _build_vitfwd_uops(ver):
    """8-stage datapath, shared by seed/steady/step states:
      s0: idx = ADD(CURR, One)              global element counter scan
      s1: a   = ADD(Src0, Src1)             T''[page, n] + v[n]
      s2: r   = MAX(CURR, a)                running page max scan
      s3: f   = IS_GE(a, r)                 achiever flag (cond for s4)
      s4: k   = SELECT(f ? idx : CURR)      last-achiever index scan
      s5: c2  = IS_LT(a, C0)                pad detector (cond for s6)
      s6: out = SELECT(c2 ? r : k)
      s7: BYPASS
    Lanes: 0: Src0 -> a, 1: Src1 -> idx, 2: One -> r, 3: C0, 4: Zero -> k.
    States: seed (idx=0, r=C0), steady, step (r = BYPASS(a) page reset).
    """
    nIdx = Scan(UAluOp.ADD, One)
    nA = Bin(UAluOp.ADD, Src0, Src1)
    nR = Scan(UAluOp.MAX, nA)
    nF = Bin(UAluOp.IS_GE, nA, nR)
    nK = Tri(UAluOp.SELECT, nF, nIdx, Zero)   # placeholder cond/operands
    nC2 = Bin(UAluOp.IS_LT, nA, C0)
    nOut = Tri(UAluOp.SELECT, nC2, nR, nK)

    pipeline = [
        _Stage(UAluOp.ADD, AluInp.CURR_ALU_OUT, One),     # s0 idx
        _Stage(UAluOp.ADD, Src0, Src1),                   # s1 a
        _Stage(UAluOp.MAX, AluInp.CURR_ALU_OUT, nA),      # s2 r (nA -> PREV)
        _Stage(UAluOp.IS_GE, nA, nR),                     # s3 f (lane0, PREV)
        _Stage(UAluOp.SELECT, AluInp.CURR_ALU_OUT, nIdx),  # s4 k
        _Stage(UAluOp.IS_LT, nA, C0),                     # s5 pad cond
        _Stage(UAluOp.SELECT, nK, nR),                    # s6 out
        _Stage(UAluOp.BYPASS, PREV),                      # s7
    ]
    node_stage = {nIdx: 0, nA: 1, nR: 2, nF: 3, nK: 4, nC2: 5, nOut: 6}
    lane = {Src0: 0, nA: 0, Src1: 1, nIdx: 1, One: 2, nR: 2, C0: 3,
            Zero: 4, nK: 4}
    captures = [(1, 1), (2, 0), (3, 2), (5, 4)]
    p = _Placement(
        pipeline=pipeline, node_stage=node_stage, lane=lane,
        out_sel=OutSel.ALU_OUT, accum_stage=None, captures=captures)

    seed = _State(
        placement=p,
        overrides={0: _Stage(UAluOp.BYPASS, Zero), 2: _Stage(UAluOp.BYPASS, C0)},
        trigger=COUNT_ONCE, repeat=1, next=(1, 0, 0), write_out=False,
        consume=(False, False))
    steady = _State(
        placement=p, consume=(True, True),
        trigger=(Trigger.SRC_TENSOR_DONE, Trigger.SUB_DIM_DONE, Trigger.NONE),
        next=(0, 2, 0))
    step = _State(
        placement=p, consume=(True, True),
        overrides={2: _Stage(UAluOp.BYPASS, PREV)},
        trigger=(Trigger.SRC_TENSOR_DONE, Trigger.SUB_DIM_DONE, Trigger.COUNT),
        next=(0, 2, 1), repeat=1)
    return [_assemble(seed), _assemble(steady), _assemble(step)]


def _register_vitfwd():
    """out[p,s,n] for pages s of width 27:
      a = in0 + in1; r = running max of a (reset per page);
      k = stream index of last position with a >= r (global counter);
      out = r where a < C0 (the pad slot), else k.
    Page slot 25 holds the first-argmax (via reversed labels), slot 26 the
    page max."""
    name = "VITFWD_ANT"
    if name in dve_ops._SUB_OPCODE_FOR_NAME:
        for op in dve_ops.OPS:
            if op.name == name:
                return op

    def _ref(in0, in1, s0, s1, imm2):
        # 1-based stream index: the uop counter seeds 0 and pre-increments,
        # so element e carries index e+1. KOFF = 27*s + 1 recovers n*.
        P = in0.shape[0]
        a = (np.asarray(in0, np.float32)
             + np.asarray(in1, np.float32)).reshape(P, -1, NPAD)
        S = a.shape[1]
        r = np.maximum.accumulate(a, axis=2)
        f = a >= r
        idxg = np.arange(1, S * NPAD + 1, dtype=np.float32).reshape(S, NPAD)
        k = np.maximum.accumulate(
            np.where(f, idxg[None], np.float32(0)).reshape(P, -1), axis=1
        ).reshape(P, S, NPAD)
        pad = (np.arange(NPAD) == NPAD - 1)[None, None, :]
        return np.where(pad, r, k).astype(np.float32).reshape(in0.shape)

    # The body is a throwaway-but-legal spec: it only defines the leaf set
    # (Src0, Src1, C0) for lowering checks; the uops come from _build above.
    spec = Spec(body=scan(UAluOp.MAX, Src0 + Src1) * (Src0 < C0),
                reference=_ref)
    opcode = max(dve_ops._SUB_OPCODE_FOR_NAME.values()) + 1
    dve_ops._SUB_OPCODE_FOR_NAME[name] = opcode
    shas = {}
    for ver in ("v3", "v4"):
        uops = _build_vitfwd_uops(ver)
        for u in uops:
            u.validate(ver)
        s = DveOpSpec(name=name, opcode=opcode, uops=uops, rd1_en=True)
        shas[ver] = s.sha(ver)
    op = _HandDveOp(name, spec, True, shas, build=_build_vitfwd_uops)
    dve_ops.OPS.append(op)
    dve_ops.CUSTOM_DVE_SPECS[name] = spec
    return op


VITFWD = _register_vitfwd()


def _register_eqsel():
    """out[p,s,n] = (n == in1[p,s,n]) * in0[p,s,n] -- one-hot select of a
    backpointer row by label index, one pass."""
    name = "EQSEL_ANT"
    if name in dve_ops._SUB_OPCODE_FOR_NAME:
        for op in dve_ops.OPS:
            if op.name == name:
                return op

    def _ref(in0, in1, s0, s1, imm2):
        N = in0.shape[-1]
        P = in0.shape[0]
        a = np.asarray(in0, np.float32).reshape(P, -1, N)
        b = np.asarray(in1, np.float32).reshape(a.shape)
        S = a.shape[1]
        n = (np.arange(S * N, dtype=np.float32)
             - np.repeat(np.arange(S), N) * s1).reshape(S, N)
        return ((n[None] == b).astype(np.float32) * a).reshape(in0.shape)

    spec = Spec(body=eq(Idx - SubIdx * C1, Src1) * Src0, reference=_ref)
    opcode = max(dve_ops._SUB_OPCODE_FOR_NAME.values()) + 1
    dve_ops._SUB_OPCODE_FOR_NAME[name] = opcode
    shas = {}
    for ver in ("v3", "v4"):
        sp = DveOpSpec(name=name, opcode=opcode, uops=lower(spec, ver=ver),
                       rd1_en=True)
        shas[ver] = sp.sha(ver)
    op = DveOp(name, spec, subdim=True, uops_sha=shas)
    dve_ops.OPS.append(op)
    dve_ops.CUSTOM_DVE_SPECS[name] = spec
    return op


EQSEL = _register_eqsel()


def _conv_matrix(K: np.ndarray) -> np.ndarray:
    """C[o, i] such that conv_SAME(x.reshape(H,WD)) flattened == C @ x."""
    K2 = K.reshape(KS, KS).astype(np.float64)
    C = np.zeros((F, F), dtype=np.float64)
    for r in range(H):
        for c in range(WD):
            o = r * WD + c
            for dy in range(KS):
                for dx in range(KS):
                    rr = r + dy - KS // 2
                    cc = c + dx - KS // 2
                    if 0 <= rr < H and 0 <= cc < WD:
                        C[o, rr * WD + cc] = K2[dy, dx]
    return C


def _consts(K, b, W, T):
    """Host-side constant tensors in primed label space l' = 25 - l
    (fp64 math, one final fp32 round)."""
    C = _conv_matrix(K)
    A = W.astype(np.float64) @ C                         # (L, F)
    c0 = float(b[0]) * W.astype(np.float64).sum(axis=1)  # (L,)
    Tp = T.astype(np.float64) + c0[None, :]              # T'[i,j]
    ATP = np.ascontiguousarray(A.T[:, ::-1]).astype(np.float32)   # (F, L)
    M2 = np.ascontiguousarray(Tp[::-1, ::-1].T)          # (26s, 26n) primed
    TTKPAD = np.broadcast_to(
        np.concatenate([M2.astype(np.float32),
                        np.full((L, 1), PAD_VAL, np.float32)], axis=1)[None],
        (128, L, NPAD)).copy()                           # (128, 26, 27)
    C0B = np.broadcast_to(
        c0[::-1].astype(np.float32)[None], (128, L)).copy()
    KOFF = np.broadcast_to(
        (NPAD * np.arange(L) + 1).astype(np.float32)[None], (128, L)).copy()
    IOTA0 = np.broadcast_to(
        np.arange(L, dtype=np.float32)[None], (128, L)).copy()
    IDN = np.eye(128, dtype=np.float32)
    return ATP, TTKPAD, C0B, KOFF, IOTA0, IDN


# DP wave sizes: each wave's tiles must be emitted before its DP starts;
# emissions pace ~3us/tile while DP consumes ~10us/tile, so waves can grow.
WAVES = [(0, 4), (4, 10), (10, 18), (18, 32)]


def build_module():
    nc = bacc.Bacc("TRN2", target_bir_lowering=False, debug=False,
                   num_devices=NCORES)
    xs = nc.dram_tensor("XS", [BC, M, F], F32, kind="ExternalInput")
    at_d = nc.dram_tensor("ATP", [F, L], F32, kind="ExternalInput")
    ttk_d = nc.dram_tensor("TTKPAD", [128, L, NPAD], F32, kind="ExternalInput")
    c0_d = nc.dram_tensor("C0B", [128, L], F32, kind="ExternalInput")
    koff_d = nc.dram_tensor("KOFF", [128, L], F32, kind="ExternalInput")
    io_d = nc.dram_tensor("IOTA0", [128, L], F32, kind="ExternalInput")
    id_d = nc.dram_tensor("IDN", [128, 128], F32, kind="ExternalInput")
    out_d = nc.dram_tensor("OUT", [BC, M], mybir.dt.int32,
                           kind="ExternalOutput")

    with tile.TileContext(nc) as tc:
        with (
            tc.tile_pool(name="const", bufs=1) as cpool,
            tc.tile_pool(name="pers", bufs=1) as ppool,
            tc.tile_pool(name="xin", bufs=3) as xpool,
            tc.tile_pool(name="xts", bufs=2) as tpool,
            tc.tile_pool(name="dp", bufs=3) as dpool,
            tc.tile_pool(name="psT", bufs=2, space="PSUM") as psT,
            tc.tile_pool(name="psS", bufs=2, space="PSUM") as psS,
        ):
            at = cpool.tile([F, L], F32)
            ttk = cpool.tile([128, L, NPAD], F32)
            c0b = cpool.tile([128, L], F32)
            koff = cpool.tile([128, L], F32)
            iota = cpool.tile([128, L], F32)
            idn = cpool.tile([128, 128], F32)
            nc.sync.dma_start(at[:], at_d.ap())
            nc.sync.dma_start(ttk[:], ttk_d.ap())
            nc.sync.dma_start(c0b[:], c0_d.ap())
            nc.sync.dma_start(koff[:], koff_d.ap())
            nc.sync.dma_start(iota[:], io_d.ap())
            nc.sync.dma_start(idn[:], id_d.ap())

            sc = ppool.tile([128, NT, M, L], F32)       # emission scores
            bp = ppool.tile([128, NT, M - 1, L], F32)   # backpointers (primed)
            vall = ppool.tile([128, NT, NPAD], F32)     # v + zero pad col
            path = ppool.tile([128, NT, M], F32)

            # col 26 of every v must be 0 forever (pad rides on TTKPAD)
            nc.gpsimd.memset(vall[:], 0.0)

            xs_t = xs.ap().rearrange("(n p) m f -> n p (m f)", p=128)

            # ---- emissions: all tiles, pipelined on DMA/PE/Act ----
            for wt in range(NT):
                xt = xpool.tile([128, M * F], F32, tag="xt")
                nc.sync.dma_start(xt[:], xs_t[wt])
                scp = psS.tile([128, M * L], F32, tag="scp")
                for q in range(4):                      # quads of letters
                    lo = q * 4
                    hi = min(lo + 4, M)
                    nlet = hi - lo
                    xT = psT.tile([128, 512], F32, tag="xT")
                    for j in range(nlet):
                        m = lo + j
                        nc.tensor.transpose(
                            xT[:, j * F:(j + 1) * F],
                            xt[:, m * F:(m + 1) * F], idn[:])
                    xTs = tpool.tile([128, 512], F32, tag="xTs")
                    nc.scalar.activation(
                        xTs[:, :nlet * F], xT[:, :nlet * F], ACT_COPY)
                    for j in range(nlet):
                        m = lo + j
                        nc.tensor.matmul(
                            scp[:, m * L:(m + 1) * L],
                            xTs[:, j * F:(j + 1) * F], at[:])
                nc.scalar.activation(
                    sc[:, wt].rearrange("p m l -> p (m l)"), scp[:], ACT_COPY)

            # ---- Viterbi DP in waves ----
            for (w0, w1) in WAVES:
                for wt in range(w0, w1):
                    nc.gpsimd.tensor_tensor(
                        vall[:, wt, :L], sc[:, wt, 0, :], c0b[:], op=OP.add)
                for t in range(1, M):
                    for wt in range(w0, w1):
                        slab = dpool.tile([128, L, NPAD], F32, tag="slab")
                        v_b = vall[:, wt, :].unsqueeze(1).broadcast_to(
                            (128, L, NPAD))
                        nc.vector._custom_dve(
                            VITFWD, out=slab[:], in0=ttk[:], in1=v_b,
                            s0=RSEED)
                        nc.gpsimd.tensor_tensor(
                            bp[:, wt, t - 1, :], slab[:, :, L - 1], koff[:],
                            op=OP.subtract)
                        nc.gpsimd.tensor_tensor(
                            vall[:, wt, :L], slab[:, :, NPAD - 1],
                            sc[:, wt, t, :], op=OP.add)

            # ---- batched backtrack over all tiles (primed space) ----
            ew = ppool.tile([128, NT, L], F32)
            rw = ppool.tile([128, NT], F32)
            io_bt = iota[:].unsqueeze(1).broadcast_to((128, NT, L))

            vf = vall[:, :, :L]
            nc.vector.tensor_reduce(rw[:], vf, axis=AX.X, op=OP.max)
            nc.vector.tensor_tensor(
                ew[:], vf, rw[:].unsqueeze(2).broadcast_to((128, NT, L)),
                op=OP.is_ge)
            nc.vector.tensor_tensor(ew[:], ew[:], io_bt, op=OP.mult)
            nc.vector.tensor_reduce(path[:, :, M - 1], ew[:], axis=AX.X,
                                    op=OP.max)
            for t in range(M - 2, -1, -1):
                nxt = path[:, :, t + 1].unsqueeze(2).broadcast_to((128, NT, L))
                nc.vector._custom_dve(
                    EQSEL, out=ew[:], in0=bp[:, :, t, :], in1=nxt, s1=float(L))
                nc.vector.tensor_reduce(path[:, :, t], ew[:], axis=AX.X,
                                        op=OP.max)

            # primed -> true labels, int convert, DMA out
            pt = ppool.tile([128, NT, M], F32)
            nc.vector.tensor_scalar(
                pt[:], path[:], -1.0, float(L - 1), op0=OP.mult, op1=OP.add)
            pi = ppool.tile([128, NT, M], mybir.dt.int32)
            nc.vector.tensor_copy(pi[:], pt[:])
            out_t = out_d.ap().rearrange("(n p) m -> p n m", p=128)
            nc.sync.dma_start(out_t, pi[:])

    nc.compile()
    return nc


_CACHE = {}


def _get_module():
    if "nc" not in _CACHE:
        _CACHE["nc"] = build_module()
    return _CACHE["nc"]


def make_in_maps(X, K, b, W, T):
    ATP, TTKPAD, C0B, KOFF, IOTA0, IDN = _consts(K, b, W, T)
    consts = {"ATP": ATP, "TTKPAD": TTKPAD, "C0B": C0B, "KOFF": KOFF,
              "IOTA0": IOTA0, "IDN": IDN}
    X = np.ascontiguousarray(X, dtype=np.float32)
    return [dict(consts, XS=X[c * BC:(c + 1) * BC]) for c in range(NCORES)]


def kernel(X, K, b, W, T):
    nc = _get_module()
    in_maps = make_in_maps(X, K, b, W, T)
    res = bass_utils.run_bass_kernel_spmd(nc, in_maps,
                                          core_ids=list(range(NCORES)))
    out = np.concatenate([res.results[c]["OUT"] for c in range(NCORES)], axis=0)
    return out.reshape(B, M, 1).astype(np.int32)


# revision 60
# speedup vs baseline: 1.0041x; 1.0041x over previous
"""CRF decode (conv features -> emission scores -> Viterbi) on 8 TRN2 cores.

Data-parallel over the batch: each core gets B/8 = 4096 words (32 tiles of
128 words on partitions). Per core:
  - conv+emission collapse to one (128 -> 26) linear map A = W @ C applied on
    the PE per 128-word tile (one PE transpose + one matmul per letter;
    PSUM->SBUF copies batched 4 letters per Activation instruction, scores
    accumulated 14 letters per PSUM tile -> one copy per tile),
  - Viterbi forward DP on the DVE with words on partitions and a single fused
    custom-DVE pass per (step, tile): pages of 27 (26 prev-labels in reversed
    "primed" order + one -inf pad). An 8-stage hand-assembled uop program
    computes a = T''+v, running page max r (reset at page boundaries via a
    SUB_DIM_DONE step state), achiever flag f = (a >= r), and a select-scan
    k = stream index of the last achiever. Output stream = k, except the pad
    slot which emits r. Page slot 25 is the first-argmax backpointer (exact
    jax tie-breaking via the label reversal), slot 26 the max. This halves
    DVE work vs separate max + argmax passes.
  - the v updates (r + score) and backpointer extraction (k - offset,
    offset-encoded per tile) run on the Pool engine (gpsimd) in the DVE's
    shadow; tiles are processed in staggered waves so the DVE never waits
    on emissions. The step-1 transition constant has the conv-bias init
    folded in so no separate v-init op exists (the per-engine counting
    semaphores would head-of-line-block the Pool queue on it),
  - batched backtrack over all tiles with a second hand-built custom op
    EQSELR (one-hot select by label + running page max in one pass; the
    page-final slice IS the reduction), final primed->true label flip
    fused with the int32 convert, split DMA out.
  - head latency: DMA issues are need-ordered (the SP sequencer costs
    ~650ns per dma_start), the first tiles' X DMAs are split per quad,
    per-quad score copies unlock early DP steps, and dummy PE transposes
    cover the tensor engine's P-state ramp.
"""

import sys

if "/opt/trn_rl_repo" not in sys.path:
    sys.path.insert(0, "/opt/trn_rl_repo")

import numpy as np

from dataclasses import dataclass
from typing import Any

import concourse.bacc as bacc
import concourse.mybir as mybir
import concourse.tile as tile
from concourse import bass_utils
from concourse import dve_ops
from concourse.dve_ops import DveOp
from concourse.dve_spec import (
    Spec, Src0, Src1, C0, C1, Zero, One, scan, Leaf,
    Bin, Tri, Scan, _Stage, _State, _Placement, _assemble, PREV, COUNT_ONCE,
)
from concourse.dve_uop import (
    AluInp, AluOp as UAluOp, DveOpSpec, InpSel, OutPath, OutSel, Trigger,
    ENABLE,
)

F32 = mybir.dt.float32
AX = mybir.AxisListType
OP = mybir.AluOpType
ACT_COPY = mybir.ActivationFunctionType.Copy

B = 32768
M = 14
H, WD = 16, 8
F = 128
L = 26
KS = 5
NCORES = 8
BC = B // NCORES          # words per core
NT = BC // 128            # 128-word tiles per core (32)
NPAD = L + 1              # 27: page width (26 prev-labels + -inf pad)
PAD_VAL = -3e38
RSEED = -1e37             # r scan seed / pad detection threshold

_HAND_CACHE: dict = {}


@dataclass(frozen=True)
class _HandDveOp(DveOp):
    """DveOp whose uops come from `build` (hand-assembled program that the
    Spec language cannot express: two interacting scans + select-routed
    output). `build2x`, when set, supplies a pair-per-cycle program for the
    2X_2PORT perf-mode table slots."""

    build: Any = None
    build2x: Any = None

    def compile(self, ver):
        key = (self.name, ver)
        if (r := _HAND_CACHE.get(key)) is not None:
            return r
        uops = self.build(ver)
        for u in uops:
            u.validate(ver)
        kw = {}
        if self.build2x is not None:
            u2 = self.build2x(ver)
            for u in u2:
                u.validate(ver)
            kw = dict(uops_2x=u2, uops_2x_2p=u2)
        res = DveOpSpec(
            name=self.name, opcode=dve_ops.get_dve_sub_opcode(self.name),
            uops=uops, rd1_en=True, **kw)
        _HAND_CACHE[key] = res
        return res


def _build_vitfwd_uops(ver):
    """8-stage datapath, shared by seed/steady/step states:
      s0: idx = ADD(CURR, One)              global element counter scan
      s1: a   = ADD(Src0, Src1)             T''[page, n] + v[n]
      s2: r   = MAX(CURR, a)                running page max scan
      s3: f   = IS_GE(a, r)                 achiever flag (cond for s4)
      s4: k   = SELECT(f ? idx : CURR)      last-achiever index scan
      s5: c2  = IS_LT(a, C0)                pad detector (cond for s6)
      s6: out = SELECT(c2 ? r : k)
      s7: BYPASS
    Lanes: 0: Src0 -> a, 1: Src1 -> idx, 2: One -> r, 3: C0, 4: Zero -> k.
    States: seed (idx=0, r=C0), steady, step (r = BYPASS(a) page reset).
    """
    nIdx = Scan(UAluOp.ADD, One)
    nA = Bin(UAluOp.ADD, Src0, Src1)
    nR = Scan(UAluOp.MAX, nA)
    nF = Bin(UAluOp.IS_GE, nA, nR)
    nK = Tri(UAluOp.SELECT, nF, nIdx, Zero)   # placeholder cond/operands
    nC2 = Bin(UAluOp.IS_LT, nA, C0)
    nOut = Tri(UAluOp.SELECT, nC2, nR, nK)

    pipeline = [
        _Stage(UAluOp.ADD, AluInp.CURR_ALU_OUT, One),     # s0 idx
        _Stage(UAluOp.ADD, Src0, Src1),                   # s1 a
        _Stage(UAluOp.MAX, AluInp.CURR_ALU_OUT, nA),      # s2 r (nA -> PREV)
        _Stage(UAluOp.IS_GE, nA, nR),                     # s3 f (lane0, PREV)
        _Stage(UAluOp.SELECT, AluInp.CURR_ALU_OUT, nIdx),  # s4 k
        _Stage(UAluOp.IS_LT, nA, C0),                     # s5 pad cond
        _Stage(UAluOp.SELECT, nK, nR),                    # s6 out
        _Stage(UAluOp.BYPASS, PREV),                      # s7
    ]
    node_stage = {nIdx: 0, nA: 1, nR: 2, nF: 3, nK: 4, nC2: 5, nOut: 6}
    lane = {Src0: 0, nA: 0, Src1: 1, nIdx: 1, One: 2, nR: 2, C0: 3,
            Zero: 4, nK: 4}
    captures = [(1, 1), (2, 0), (3, 2), (5, 4)]
    p = _Placement(
        pipeline=pipeline, node_stage=node_stage, lane=lane,
        out_sel=OutSel.ALU_OUT, accum_stage=None, captures=captures)

    seed = _State(
        placement=p,
        overrides={0: _Stage(UAluOp.BYPASS, Zero), 2: _Stage(UAluOp.BYPASS, C0)},
        trigger=COUNT_ONCE, repeat=1, next=(1, 0, 0), write_out=False,
        consume=(False, False))
    steady = _State(
        placement=p, consume=(True, True),
        trigger=(Trigger.SRC_TENSOR_DONE, Trigger.SUB_DIM_DONE, Trigger.NONE),
        next=(0, 2, 0))
    step = _State(
        placement=p, consume=(True, True),
        overrides={2: _Stage(UAluOp.BYPASS, PREV)},
        trigger=(Trigger.SRC_TENSOR_DONE, Trigger.SUB_DIM_DONE, Trigger.COUNT),
        next=(0, 2, 1), repeat=1)
    return [_assemble(seed), _assemble(steady), _assemble(step)]


def _register_vitfwd():
    """out[p,s,n] for pages s of width 27:
      a = in0 + in1; r = running max of a (reset per page);
      k = stream index of last position with a >= r (global counter);
      out = r where a < C0 (the pad slot), else k.
    Page slot 25 holds the first-argmax (via reversed labels), slot 26 the
    page max."""
    name = "VITFWD_ANT"
    if name in dve_ops._SUB_OPCODE_FOR_NAME:
        for op in dve_ops.OPS:
            if op.name == name:
                return op

    def _ref(in0, in1, s0, s1, imm2):
        # 1-based stream index: the uop counter seeds 0 and pre-increments,
        # so element e carries index e+1. KOFF = 27*s + 1 recovers n*.
        P = in0.shape[0]
        a = (np.asarray(in0, np.float32)
             + np.asarray(in1, np.float32)).reshape(P, -1, NPAD)
        S = a.shape[1]
        r = np.maximum.accumulate(a, axis=2)
        f = a >= r
        idxg = np.arange(1, S * NPAD + 1, dtype=np.float32).reshape(S, NPAD)
        k = np.maximum.accumulate(
            np.where(f, idxg[None], np.float32(0)).reshape(P, -1), axis=1
        ).reshape(P, S, NPAD)
        pad = (np.arange(NPAD) == NPAD - 1)[None, None, :]
        return np.where(pad, r, k).astype(np.float32).reshape(in0.shape)

    # The body is a throwaway-but-legal spec: it only defines the leaf set
    # (Src0, Src1, C0) for lowering checks; the uops come from _build above.
    spec = Spec(body=scan(UAluOp.MAX, Src0 + Src1) * (Src0 < C0),
                reference=_ref)
    opcode = max(dve_ops._SUB_OPCODE_FOR_NAME.values()) + 1
    dve_ops._SUB_OPCODE_FOR_NAME[name] = opcode
    shas = {}
    for ver in ("v3", "v4"):
        uops = _build_vitfwd_uops(ver)
        for u in uops:
            u.validate(ver)
        s = DveOpSpec(name=name, opcode=opcode, uops=uops, rd1_en=True)
        shas[ver] = s.sha(ver)
    op = _HandDveOp(name, spec, True, shas, build=_build_vitfwd_uops)
    dve_ops.OPS.append(op)
    dve_ops.CUSTOM_DVE_SPECS[name] = spec
    return op


VITFWD = _register_vitfwd()


def _build_eqselr_uops(ver):
    """4-stage datapath:
      s0: w = ADD(CURR, One)      within-page index (seed -1, step reset 0)
      s1: e = IS_EQ(w, Src1)      one-hot match against the lookup label
      s2: m = MUL(e, Src0)        select the bp value
      s3: k = MAX(CURR, m)        running page max (seed 0, step BYPASS)
    Page-final slice = the selected value (bp values are >= 0)."""
    pipeline = [
        _Stage(UAluOp.ADD, AluInp.CURR_ALU_OUT, One),
        _Stage(UAluOp.IS_EQ, PREV, Src1),
        _Stage(UAluOp.MULTIPLY, PREV, Src0),
        _Stage(UAluOp.MAX, AluInp.CURR_ALU_OUT, PREV),
        _Stage(UAluOp.BYPASS, PREV),
        _Stage(UAluOp.BYPASS, PREV),
        _Stage(UAluOp.BYPASS, PREV),
        _Stage(UAluOp.BYPASS, PREV),
    ]
    lane = {One: 0, Src1: 1, Src0: 2, Zero: 3}
    p = _Placement(
        pipeline=pipeline, node_stage={}, lane=lane,
        out_sel=OutSel.ALU_OUT, accum_stage=None, captures=[])
    seed = _State(
        placement=p,
        overrides={0: _Stage(UAluOp.SUBTRACT, Zero, One),
                   3: _Stage(UAluOp.BYPASS, Zero)},
        trigger=COUNT_ONCE, repeat=1, next=(1, 0, 0), write_out=False,
        consume=(False, False))
    steady = _State(
        placement=p, consume=(True, True),
        trigger=(Trigger.SRC_TENSOR_DONE, Trigger.SUB_DIM_DONE, Trigger.NONE),
        next=(0, 2, 0))
    step = _State(
        placement=p, consume=(True, True),
        overrides={0: _Stage(UAluOp.BYPASS, Zero),
                   3: _Stage(UAluOp.BYPASS, PREV)},
        trigger=(Trigger.SRC_TENSOR_DONE, Trigger.SUB_DIM_DONE, Trigger.COUNT),
        next=(0, 2, 1), repeat=1)
    return [_assemble(seed), _assemble(steady), _assemble(step)]


def _build_eqselr_uops_2x(ver):
    """Pair-per-cycle (2X_2PORT) variant: elements lo=2c / hi=2c+1 arrive
    together on the SRC_*_LO/HI lanes. Pages are 26 elements = 13 pairs,
    so SUB_DIM boundaries stay pair-aligned and no pad is needed.
      s0: w_lo = ADD(CURR, C0=2)    [step: C1-C1 = 0]
      s1: w_hi = ADD(CURR, C0=2)    [step: BYPASS(C1=1)]
      s2: e_lo = IS_EQ(w_lo, S1lo)
      s3: e_hi = IS_EQ(w_hi, S1hi)
      s4: m_lo = MUL(e_lo, S0lo)
      s5: m_hi = MUL(e_hi, S0hi)
      s6: mm   = MAX(m_lo, m_hi)
      s7: K    = MAX(CURR, mm)      [step: BYPASS(mm)]
    Both write paths emit K; only HI slots are read downstream (slot 25),
    where K is the completed post-pair value. The seed state is a no-op
    garbage pass -- it jumps to the step state, which writes the counters
    absolutely."""
    S0H, S1H = Leaf(InpSel.SRC_0_HI), Leaf(InpSel.SRC_1_HI)
    # structurally-distinct sentinels (Bin is a value-equal dataclass)
    nWlo = Bin(UAluOp.ADD, Zero, One)
    nWhi = Bin(UAluOp.ADD, One, Zero)
    nElo = Bin(UAluOp.IS_EQ, Zero, One)
    nEhi = Bin(UAluOp.IS_EQ, One, Zero)
    nMlo = Bin(UAluOp.MULTIPLY, Zero, One)
    nMhi = Bin(UAluOp.MULTIPLY, One, Zero)
    nMm = Bin(UAluOp.MAX, Zero, One)
    pipeline = [
        _Stage(UAluOp.ADD, AluInp.CURR_ALU_OUT, C0),
        _Stage(UAluOp.ADD, AluInp.CURR_ALU_OUT, C0),
        _Stage(UAluOp.IS_EQ, nWlo, Src1),
        _Stage(UAluOp.IS_EQ, nWhi, S1H),
        _Stage(UAluOp.MULTIPLY, nElo, Src0),
        _Stage(UAluOp.MULTIPLY, nEhi, S0H),
        _Stage(UAluOp.MAX, nMlo, nMhi),
        _Stage(UAluOp.MAX, AluInp.CURR_ALU_OUT, nMm),
    ]
    node_stage = {nWlo: 0, nWhi: 1, nElo: 2, nEhi: 3, nMlo: 4, nMhi: 5,
                  nMm: 6}
    lane = {Src0: 0, nMlo: 0, S0H: 1, Src1: 2, nElo: 2, S1H: 3, nEhi: 3,
            C0: 4, nWhi: 4, C1: 5, nWlo: 5}
    captures = [(1, 5), (2, 4), (3, 2), (4, 3), (5, 0)]
    p = _Placement(
        pipeline=pipeline, node_stage=node_stage, lane=lane,
        out_sel=OutSel.ALU_OUT, accum_stage=None, captures=captures)
    seed = _State(
        placement=p, overrides={}, trigger=COUNT_ONCE, repeat=1,
        next=(2, 0, 0), write_out=False, consume=(False, False))
    steady = _State(
        placement=p, consume=(True, True),
        trigger=(Trigger.SRC_TENSOR_DONE, Trigger.SUB_DIM_DONE, Trigger.NONE),
        next=(0, 2, 0))
    step = _State(
        placement=p, consume=(True, True),
        overrides={0: _Stage(UAluOp.SUBTRACT, C1, C1),
                   1: _Stage(UAluOp.BYPASS, C1),
                   7: _Stage(UAluOp.BYPASS, PREV)},
        trigger=(Trigger.SRC_TENSOR_DONE, Trigger.SUB_DIM_DONE, Trigger.COUNT),
        next=(0, 2, 1), repeat=1)
    uops = [_assemble(seed), _assemble(steady), _assemble(step)]
    for u in uops[1:]:                       # emit both halves of each pair
        u.out[OutPath.WR0_HI] = OutSel.ALU_OUT
        u.out_enable[OutPath.WR0_HI] = ENABLE
    return uops


def _register_eqselr():
    """out[p,s,n] = running page max of (n == in1[p,s,n]) * in0[p,s,n] --
    one-hot select of a backpointer row by label index; the page-final
    slice [:, :, N-1] is the selected value (a fused select + reduce)."""
    name = "EQSELR_ANT"
    if name in dve_ops._SUB_OPCODE_FOR_NAME:
        for op in dve_ops.OPS:
            if op.name == name:
                return op

    def _ref(in0, in1, s0, s1, imm2):
        # page width is always L here (s0/s1 carry the 2x counter consts)
        N = in0.shape[-1]
        P = in0.shape[0]
        a = np.asarray(in0, np.float32).reshape(P, -1, N)
        b = np.asarray(in1, np.float32).reshape(a.shape)
        S = a.shape[1]
        n = np.broadcast_to(np.arange(N, dtype=np.float32), (S, N))
        sel = (n[None] == b).astype(np.float32) * a
        return np.maximum.accumulate(sel, axis=2).reshape(in0.shape)

    spec = Spec(body=scan(UAluOp.MAX, (Src0 + Src1) * C1), reference=_ref)
    opcode = max(dve_ops._SUB_OPCODE_FOR_NAME.values()) + 1
    dve_ops._SUB_OPCODE_FOR_NAME[name] = opcode
    shas = {}
    for ver in ("v3", "v4"):
        uops = _build_eqselr_uops(ver)
        u2 = _build_eqselr_uops_2x(ver)
        for u in uops + u2:
            u.validate(ver)
        s = DveOpSpec(name=name, opcode=opcode, uops=uops, uops_2x=u2,
                      uops_2x_2p=u2, rd1_en=True)
        shas[ver] = s.sha(ver)
    op = _HandDveOp(name, spec, True, shas, build=_build_eqselr_uops,
                    build2x=_build_eqselr_uops_2x)
    dve_ops.OPS.append(op)
    dve_ops.CUSTOM_DVE_SPECS[name] = spec
    return op


EQSELR = _register_eqselr()


def _conv_matrix(K: np.ndarray) -> np.ndarray:
    """C[o, i] such that conv_SAME(x.reshape(H,WD)) flattened == C @ x."""
    K2 = K.reshape(KS, KS).astype(np.float64)
    C = np.zeros((F, F), dtype=np.float64)
    for r in range(H):
        for c in range(WD):
            o = r * WD + c
            for dy in range(KS):
                for dx in range(KS):
                    rr = r + dy - KS // 2
                    cc = c + dx - KS // 2
                    if 0 <= rr < H and 0 <= cc < WD:
                        C[o, rr * WD + cc] = K2[dy, dx]
    return C


def _consts(K, b, W, T):
    """Host-side constant tensors in primed label space l' = 25 - l
    (fp64 math, one final fp32 round)."""
    C = _conv_matrix(K)
    A = W.astype(np.float64) @ C                         # (L, F)
    c0 = float(b[0]) * W.astype(np.float64).sum(axis=1)  # (L,)
    Tp = T.astype(np.float64) + c0[None, :]              # T'[i,j]
    ATP = np.ascontiguousarray(A.T[:, ::-1]).astype(np.float32)   # (F, L)
    M2 = np.ascontiguousarray(Tp[::-1, ::-1].T)          # (26s, 26n) primed
    TTKPAD = np.broadcast_to(
        np.concatenate([M2.astype(np.float32),
                        np.full((L, 1), PAD_VAL, np.float32)], axis=1)[None],
        (128, L, NPAD)).copy()                           # (128, 26, 27)
    # step-1 variant with the conv-bias init c0'[n] folded in, so v_0 is
    # the raw score row and no separate v-init op is needed
    M2C = M2 + c0[::-1][None, :]
    TTKC = np.broadcast_to(
        np.concatenate([M2C.astype(np.float32),
                        np.full((L, 1), PAD_VAL, np.float32)], axis=1)[None],
        (128, L, NPAD)).copy()
    # bp is stored offset-encoded: bp_enc(wt) = n* + L*wt, strictly
    # increasing across tiles so a running-max scan can act as a segmented
    # reduce on the Pool engine. KOFFT folds the decode of the fused op's
    # k value (27*s + 1) together with the +L*wt encoding.
    wt_i = np.arange(NT)[:, None]
    s_i = np.arange(L)[None, :]
    KOFFT = np.broadcast_to(
        (NPAD * s_i + 1 - L * wt_i).astype(np.float32)[None],
        (128, NT, L)).copy()
    GOFFT = np.broadcast_to(
        (L * np.arange(NT)).astype(np.float32)[None], (128, NT)).copy()
    IOTA0 = np.broadcast_to(
        np.arange(L, dtype=np.float32)[None], (128, L)).copy()
    IDN = np.eye(128, dtype=np.float32)
    return ATP, TTKPAD, TTKC, KOFFT, GOFFT, IOTA0, IDN


# DP wave sizes: each wave's tiles must be emitted before its DP starts;
# emissions pace ~3us/tile while DP consumes ~10us/tile, so waves can grow.
# Early waves are narrow to start the DVE as soon as the first tiles land;
# the last wave is small so its DVE backtrack tail is short.
WAVES = [(0, 2), (2, 5), (5, 10), (10, 18), (18, 26), (26, 32)]


def build_module():
    nc = bacc.Bacc("TRN2", target_bir_lowering=False, debug=False,
                   num_devices=NCORES)
    xs = nc.dram_tensor("XS", [BC, M, F], F32, kind="ExternalInput")
    at_d = nc.dram_tensor("ATP", [F, L], F32, kind="ExternalInput")
    ttk_d = nc.dram_tensor("TTKPAD", [128, L, NPAD], F32, kind="ExternalInput")
    ttkc_d = nc.dram_tensor("TTKC", [128, L, NPAD], F32, kind="ExternalInput")
    koff_d = nc.dram_tensor("KOFFT", [128, NT, L], F32, kind="ExternalInput")
    goff_d = nc.dram_tensor("GOFFT", [128, NT], F32, kind="ExternalInput")
    io_d = nc.dram_tensor("IOTA0", [128, L], F32, kind="ExternalInput")
    id_d = nc.dram_tensor("IDN", [128, 128], F32, kind="ExternalInput")
    out_d = nc.dram_tensor("OUT", [BC, M], mybir.dt.int32,
                           kind="ExternalOutput")

    with tile.TileContext(nc) as tc:
        with (
            tc.tile_pool(name="const", bufs=1) as cpool,
            tc.tile_pool(name="pers", bufs=1) as ppool,
            tc.tile_pool(name="xin", bufs=3) as xpool,
            tc.tile_pool(name="xts", bufs=2) as tpool,
            tc.tile_pool(name="dp", bufs=4) as dpool,
            tc.tile_pool(name="psT", bufs=2, space="PSUM") as psT,
            tc.tile_pool(name="psS", bufs=2, space="PSUM") as psS,
        ):
            at = cpool.tile([F, L], F32)
            ttk = cpool.tile([128, L, NPAD], F32)
            ttkc = cpool.tile([128, L, NPAD], F32)
            koff = cpool.tile([128, NT, L], F32)
            goff = cpool.tile([128, NT], F32)
            iota = cpool.tile([128, L], F32)
            idn = cpool.tile([128, 128], F32)

            sc = ppool.tile([128, NT, M, L], F32)       # emission scores
            bp = ppool.tile([128, NT, M - 1, L], F32)   # backpointers (primed)
            # per-word-tile v state: separate tiles so the framework's
            # whole-tile dependency tracking doesn't serialize sweep wraps
            # (VITFWD(k, t+1) must not wait on other tiles' Pool v-updates)
            vt = [ppool.tile([128, NPAD], F32, name=f"vt{k}")
                  for k in range(NT)]
            vfinal = ppool.tile([128, NT, L], F32)      # gathered for backtrack

            xs_t = xs.ap().rearrange("(n p) m f -> n p (m f)", p=128)
            QB = [0, 4 * F, 8 * F, 12 * F, M * F]   # quad column bounds

            def issue_x(wt, chunks, eng=None):
                """DMA tile wt's letters in `chunks` pieces (per-quad tiles
                for the head tiles, halves otherwise); returns per-quad
                source lookup."""
                parts = []
                issue = (eng or nc.sync).dma_start
                bounds = QB if chunks == 4 else [0, 8 * F, M * F]
                for i in range(len(bounds) - 1):
                    lo, hi = bounds[i], bounds[i + 1]
                    xq = xpool.tile([128, hi - lo], F32, tag=f"xq{i}",
                                    name=f"xq{wt}_{i}")
                    issue(xq[:], xs_t[wt][:, lo:hi])
                    parts.append((lo, hi, xq))

                def src(m):
                    c = m * F
                    for (lo, hi, xq) in parts:
                        if lo <= c < hi:
                            return xq[:, c - lo:c - lo + F]
                    raise AssertionError
                return src

            def emit_quad(wt, q, src, scp):
                lo = q * 4
                hi = min(lo + 4, M)
                nlet = hi - lo
                xT = psT.tile([128, 512], F32, tag="xT")
                for j in range(nlet):
                    nc.tensor.transpose(
                        xT[:, j * F:(j + 1) * F], src(lo + j), idn[:])
                xTs = tpool.tile([128, 512], F32, tag="xTs")
                nc.scalar.activation(
                    xTs[:, :nlet * F], xT[:, :nlet * F], ACT_COPY)
                for j in range(nlet):
                    m = lo + j
                    nc.tensor.matmul(
                        scp[:, m * L:(m + 1) * L],
                        xTs[:, j * F:(j + 1) * F], at[:])
                # per-quad score copy: the DP step t only needs score
                # row t, so early steps unlock before the tile finishes
                nc.scalar.activation(
                    sc[:, wt, lo:hi, :].rearrange("p m l -> p (m l)"),
                    scp[:, lo * L:hi * L], ACT_COPY)

            # DMA issue order: the SP sequencer serializes dma_starts at
            # ~650ns each, so transfers are ordered by when they are first
            # needed: identity + first score quad + step-1 constants, then
            # the rest interleaved (goff/iota only at backtrack).
            nc.sync.dma_start(idn[:], id_d.ap())
            xsrc = {0: issue_x(0, 4)}
            nc.sync.dma_start(at[:], at_d.ap())
            nc.sync.dma_start(ttkc[:], ttkc_d.ap())
            nc.sync.dma_start(ttk[:], ttk_d.ap())
            xsrc[1] = issue_x(1, 4)
            nc.sync.dma_start(koff[:], koff_d.ap())
            xsrc[2] = issue_x(2, 2)
            nc.sync.dma_start(goff[:], goff_d.ap())
            nc.sync.dma_start(iota[:], io_d.ap())

            # PE warm-up: the tensor engine ramps to full clock only after
            # ~3us of continuous work; dummy transposes on a zeroed tile
            # cover the ramp so the first real emission runs at speed
            warm = cpool.tile([128, 128], F32)
            warmp = psT.tile([128, 128], F32, tag="warm")
            nc.gpsimd.memset(warm[:], 0.0)
            for _ in range(24):
                nc.tensor.transpose(warmp[:], warm[:], warm[:])

            # col 26 of every v must be 0 forever (pad rides on TTKPAD)
            for k in range(NT):
                nc.gpsimd.memset(vt[k][:, L:], 0.0)

            # ---- emissions: all tiles, pipelined on DMA/PE/Act.
            # The first two tiles' quads are interleaved so wave 0's
            # second tile is ready right after the first.
            scp01 = {wt: psS.tile([128, M * L], F32, tag="scp",
                                  name=f"scp0{wt}") for wt in (0, 1)}
            for q in range(4):
                for wt in (0, 1):
                    emit_quad(wt, q, xsrc[wt], scp01[wt])
            for wt in range(2, NT):
                src = xsrc[wt] if wt in xsrc else issue_x(wt, 2)
                scp = psS.tile([128, M * L], F32, tag="scp")
                for q in range(4):
                    emit_quad(wt, q, src, scp)

            # ---- Viterbi DP in waves ----
            for wi, (w0, w1) in enumerate(WAVES):
                for t in range(1, M):
                    for wt in range(w0, w1):
                        slab = dpool.tile([128, L, NPAD], F32, tag="slab")
                        if t == 1:
                            # v_0 = score row 0 (c0 is folded into TTKC);
                            # the 27th in1 column reads sc[1,0] -- finite
                            # garbage masked by the -inf pad in TTKC
                            scf = sc[:, wt].rearrange("p m l -> p (m l)")
                            v_b = scf[:, :NPAD].unsqueeze(1).broadcast_to(
                                (128, L, NPAD))
                            tk = ttkc
                        else:
                            v_b = vt[wt][:].unsqueeze(1).broadcast_to(
                                (128, L, NPAD))
                            tk = ttk
                        nc.vector._custom_dve(
                            VITFWD, out=slab[:], in0=tk[:], in1=v_b,
                            s0=RSEED)
                        # v update first: it is the critical path to the
                        # next step's VITFWD; the bp extraction can lag
                        nc.gpsimd.tensor_tensor(
                            vt[wt][:, :L], slab[:, :, NPAD - 1],
                            sc[:, wt, t, :], op=OP.add)
                        nc.gpsimd.tensor_tensor(
                            bp[:, wt, t - 1, :], slab[:, :, L - 1],
                            koff[:, wt, :], op=OP.subtract)
                for wt in range(w0, w1):
                    nc.gpsimd.tensor_copy(vfinal[:, wt, :], vt[wt][:, :L])

            # ---- batched backtrack over all tiles (primed space); bp is
            # offset-encoded (n* + L*wt), decoded with GOFFT after reduce
            ew = ppool.tile([128, NT, L], F32)
            rw = ppool.tile([128, NT], F32)
            path = ppool.tile([128, NT, M], F32)
            io_bt = iota[:].unsqueeze(1).broadcast_to((128, NT, L))

            vf = vfinal[:]
            nc.vector.tensor_reduce(rw[:], vf, axis=AX.X, op=OP.max)
            nc.vector.tensor_tensor(
                ew[:], vf, rw[:].unsqueeze(2).broadcast_to((128, NT, L)),
                op=OP.is_ge)
            nc.vector.tensor_tensor(ew[:], ew[:], io_bt, op=OP.mult)
            nc.vector.tensor_reduce(path[:, :, M - 1], ew[:], axis=AX.X,
                                    op=OP.max)
            for t in range(M - 2, -1, -1):
                nxt = path[:, :, t + 1].unsqueeze(2).broadcast_to((128, NT, L))
                bi = nc.vector._custom_dve(
                    EQSELR, out=ew[:], in0=bp[:, :, t, :], in1=nxt,
                    s0=2.0, s1=1.0)
                bi.ins.perf_max = 2      # 2X_2PORT eligible (fp32 in SBUF)
                # the running-max page-final slice IS the selected bp value
                nc.vector.tensor_tensor(path[:, :, t], ew[:, :, L - 1],
                                        goff[:], op=OP.subtract)
            # primed -> true labels fused with the int32 convert, then DMA
            # out (two DMAs -> parallel queues, shorter tail)
            pi = ppool.tile([128, NT, M], mybir.dt.int32)
            nc.vector.tensor_scalar(
                pi[:], path[:], -1.0, float(L - 1), op0=OP.mult, op1=OP.add)
            out_t = out_d.ap().rearrange("(n p) m -> p n m", p=128)
            nc.sync.dma_start(out_t[:, :NT // 2], pi[:, :NT // 2])
            nc.sync.dma_start(out_t[:, NT // 2:], pi[:, NT // 2:])

    nc.compile()
    return nc


_CACHE = {}


def _get_module():
    if "nc" not in _CACHE:
        _CACHE["nc"] = build_module()
    return _CACHE["nc"]


def make_in_maps(X, K, b, W, T):
    ATP, TTKPAD, TTKC, KOFFT, GOFFT, IOTA0, IDN = _consts(K, b, W, T)
    consts = {"ATP": ATP, "TTKPAD": TTKPAD, "TTKC": TTKC, "KOFFT": KOFFT,
              "GOFFT": GOFFT, "IOTA0": IOTA0, "IDN": IDN}
    X = np.ascontiguousarray(X, dtype=np.float32)
    return [dict(consts, XS=X[c * BC:(c + 1) * BC]) for c in range(NCORES)]


def kernel(X, K, b, W, T):
    nc = _get_module()
    in_maps = make_in_maps(X, K, b, W, T)
    res = bass_utils.run_bass_kernel_spmd(nc, in_maps,
                                          core_ids=list(range(NCORES)))
    out = np.concatenate([res.results[c]["OUT"] for c in range(NCORES)], axis=0)
    return out.reshape(B, M, 1).astype(np.int32)
